# revision 1
# baseline (speedup 1.0000x reference)
"""Trainium2 Bass kernel for nn_DNMPScene (PyTorch3D-style ray attribute
interpolation + depth sort).

Strategy (per spec sharding_hint: shard rays, replicate the face-attribute
table):
  Host marshaling:
    - Build the replicated face-attribute table ftab[f] =
      [feat|verts|normals for the 3 vertices of face f] (90 floats), padded
      to 128 floats (512 B rows -- required by the dma_gather 256B-multiple
      constraint and full-rate SDMA descriptors).
    - Pixel-level prep (ray-independent): apply the -1 fallback, zero-mask
      the barycentrics, clamp the face ids, and stable-sort the K=4 slots of
      every pixel by depth. This is O(P) index prep; the heavy gather stays
      on device.
    - Shard rays across 8 cores. Per core, bucket the 131072 (ray,k) slots
      by face-table chunk (dma_gather indices are int16, so the 800000-row
      table is gathered as 25 chunks of 32768 rows).
  Device (per core):
    - 25 dma_gather calls pull 512B face rows (~137K rows, ~67 MB) from HBM.
    - DVE barycentric reduce with broadcast access patterns:
      out30 = sum_v bary[s,v] * ftabrow[s, v*30:(v+1)*30].
    - Contiguous stores of tex/pts/nrm in slot order.
  Host: un-permute slots back to (ray, k) order; depth output is pure
  pixel-level data (no vertex gather) and is assembled host-side.
"""

import os
import sys

for _p in ("/opt/trn_rl_repo", "/root/.axon_site/_ro/trn_rl_repo", "/root/.axon_site"):
    if os.path.isdir(_p) and _p not in sys.path:
        sys.path.append(_p)

import numpy as np

R = 262144          # rays
P = 640000          # pixels
F = 800000          # faces
V = 420000          # vertices
D = 24              # feature dim
K = 4               # faces per pixel
NCORES = 8
RPC = R // NCORES   # rays per core
SLOTS = RPC * K     # (ray, k) slots per core
CH = 32768          # face-table chunk rows (int16 index range)
NCH = (F + CH - 1) // CH
EW = 128            # ftab row width in floats (512 B)

_compiled_cache = {}


def _build_program(caps):
    """Build the SPMD Bass program for the given per-chunk slot capacities."""
    from concourse import bacc, mybir
    import concourse.tile as tile

    btot = sum(caps) // 128
    iw = sum(caps) // 16

    nc = bacc.Bacc("TRN2", target_bir_lowering=False, debug=True)
    ftab_d = nc.dram_tensor("ftab", [F, EW], mybir.dt.float32, kind="ExternalInput")
    idx_d = nc.dram_tensor("idx16", [128, iw], mybir.dt.int16, kind="ExternalInput")
    bary_d = nc.dram_tensor("baryt", [128, btot * 3], mybir.dt.float32, kind="ExternalInput")
    texo = nc.dram_tensor("texo", [128, btot * D], mybir.dt.float32, kind="ExternalOutput")
    ptso = nc.dram_tensor("ptso", [128, btot * 3], mybir.dt.float32, kind="ExternalOutput")
    nrmo = nc.dram_tensor("nrmo", [128, btot * 3], mybir.dt.float32, kind="ExternalOutput")

    mult = mybir.AluOpType.mult
    add = mybir.AluOpType.add

    with tile.TileContext(nc) as tc:
        with (
            tc.tile_pool(name="iop", bufs=1) as iop,
            tc.tile_pool(name="gat", bufs=3) as gat,
            tc.tile_pool(name="red", bufs=2) as red,
        ):
            idx_t = iop.tile([128, iw], mybir.dt.int16)
            bary_t = iop.tile([128, btot * 3], mybir.dt.float32)
            nc.sync.dma_start(out=idx_t[:], in_=idx_d[:])
            nc.sync.dma_start(out=bary_t[:], in_=bary_d[:])
            bv3 = bary_t[:].rearrange("p (b t) -> p b t", t=3)

            offb = 0
            for c, cap in enumerate(caps):
                if cap == 0:
                    continue
                bc = cap // 128
                rows = min(CH, F - c * CH)
                fraw = gat.tile([128, bc * EW], mybir.dt.float32, tag="fraw")
                nc.gpsimd.dma_gather(
                    out_ap=fraw[:].rearrange("p (b e) -> p b e", e=EW),
                    in_ap=ftab_d[c * CH:c * CH + rows, :],
                    idxs_ap=idx_t[:, offb * 8:(offb + bc) * 8],
                    num_idxs=cap,
                    num_idxs_reg=cap,
                    elem_size=EW,
                    single_packet=False,
                )
                texp = red.tile([128, bc * D], mybir.dt.float32, tag="tex")
                tmpp = red.tile([128, bc * D], mybir.dt.float32, tag="tmp")
                ptsp = red.tile([128, bc * 3], mybir.dt.float32, tag="pts")
                tm3p = red.tile([128, bc * 3], mybir.dt.float32, tag="tm3")
                nrmp = red.tile([128, bc * 3], mybir.dt.float32, tag="nrm")
                frv = fraw[:].rearrange("p (b e) -> p b e", e=EW)
                texv = texp[:].rearrange("p (b e) -> p b e", e=D)
                tmpv = tmpp[:].rearrange("p (b e) -> p b e", e=D)
                ptsv = ptsp[:].rearrange("p (b e) -> p b e", e=3)
                tm3v = tm3p[:].rearrange("p (b e) -> p b e", e=3)
                nrmv = nrmp[:].rearrange("p (b e) -> p b e", e=3)
                for v in range(3):
                    bs = bv3[:, offb:offb + bc, v:v + 1]
                    btex = bs.broadcast_to([128, bc, D])
                    b3 = bs.broadcast_to([128, bc, 3])
                    ftex = frv[:, :, v * 30:v * 30 + D]
                    fpts = frv[:, :, v * 30 + D:v * 30 + D + 3]
                    fnrm = frv[:, :, v * 30 + D + 3:v * 30 + D + 6]
                    if v == 0:
                        nc.vector.tensor_tensor(out=texv, in0=ftex, in1=btex, op=mult)
                        nc.vector.tensor_tensor(out=ptsv, in0=fpts, in1=b3, op=mult)
                        nc.vector.tensor_tensor(out=nrmv, in0=fnrm, in1=b3, op=mult)
                    else:
                        nc.vector.tensor_tensor(out=tmpv, in0=ftex, in1=btex, op=mult)
                        nc.vector.tensor_tensor(out=texv, in0=texv, in1=tmpv, op=add)
                        nc.vector.tensor_tensor(out=tm3v, in0=fpts, in1=b3, op=mult)
                        nc.vector.tensor_tensor(out=ptsv, in0=ptsv, in1=tm3v, op=add)
                        nc.vector.tensor_tensor(out=tm3v, in0=fnrm, in1=b3, op=mult)
                        nc.vector.tensor_tensor(out=nrmv, in0=nrmv, in1=tm3v, op=add)
                nc.sync.dma_start(out=texo[:, offb * D:(offb + bc) * D], in_=texp[:])
                nc.sync.dma_start(out=ptso[:, offb * 3:(offb + bc) * 3], in_=ptsp[:])
                nc.sync.dma_start(out=nrmo[:, offb * 3:(offb + bc) * 3], in_=nrmp[:])
                offb += bc
    nc.finalize()
    return nc, btot


def _host_prep(inputs):
    """Pixel-level fallback + mask + stable depth sort; build ftab."""
    bary = np.ascontiguousarray(np.asarray(inputs["bary_coords"], dtype=np.float32))
    zbuf = np.ascontiguousarray(np.asarray(inputs["zbuf"], dtype=np.float32))
    verts = np.asarray(inputs["verts"], dtype=np.float32)
    feat = np.asarray(inputs["verts_features"], dtype=np.float32)
    vn = np.asarray(inputs["verts_normals"], dtype=np.float32)
    p2f = np.asarray(inputs["pix_to_face"], dtype=np.int32)
    faces = np.asarray(inputs["faces"], dtype=np.int32)

    vtab = np.concatenate([feat, verts, vn], axis=1).astype(np.float32)   # [V, 30]
    ftab = np.zeros((F, EW), np.float32)
    ftab[:, :90] = vtab[faces.reshape(-1)].reshape(F, 90)

    invalid = p2f == -1
    p2f_f = np.where(invalid, p2f[:, :1], p2f)
    bary_f = np.where(invalid[..., None], bary[:, :1, :], bary)
    depth_f = np.where(invalid, zbuf[:, :1], zbuf)
    validm = (p2f_f >= 0).astype(np.float32)
    bary_z = bary_f * validm[..., None]
    fidx = np.maximum(p2f_f, 0).astype(np.int32)

    order = np.argsort(depth_f, axis=1, kind="stable")
    fidx_s = np.take_along_axis(fidx, order, axis=1)
    bary_s = np.take_along_axis(bary_z, order[..., None], axis=1)
    depth_s = np.take_along_axis(depth_f, order, axis=1)
    return ftab, fidx_s, bary_s, depth_s


def kernel(**inputs):
    pix = np.asarray(inputs["pix_coords"], dtype=np.int32)
    ftab, fidx_s, bary_s, depth_s = _host_prep(inputs)

    # per-core slot bucketing by face-table chunk
    core_data = []
    counts = np.zeros((NCORES, NCH), np.int64)
    for c in range(NCORES):
        cpix = pix[c * RPC:(c + 1) * RPC]
        fidx_slot = fidx_s[cpix].reshape(-1)        # [SLOTS]
        bary_slot = bary_s[cpix].reshape(-1, 3)     # [SLOTS, 3]
        chunk = fidx_slot >> 15
        so = np.argsort(chunk, kind="stable")       # slot processing order
        counts[c] = np.bincount(chunk, minlength=NCH)
        core_data.append((fidx_slot, bary_slot, chunk, so))

    caps = tuple(int(-(-int(m) // 128) * 128) for m in counts.max(axis=0))
    btot = sum(caps) // 128
    iw = sum(caps) // 16
    off_pad = np.concatenate([[0], np.cumsum(caps)])[:NCH]

    in_maps = []
    unperm = []
    for c in range(NCORES):
        fidx_slot, bary_slot, chunk, so = core_data[c]
        cs = chunk[so]
        # position of each sorted slot within its chunk
        cstart = np.concatenate([[0], np.cumsum(counts[c])])[:NCH]
        pos = np.arange(SLOTS) - cstart[cs]
        g = off_pad[cs] + pos                        # padded global slot id
        loc = (fidx_slot[so] - cs * CH).astype(np.int16)

        idx16 = np.zeros((16, iw), np.int16)
        idx16[(pos % 16).astype(np.int64), (off_pad[cs] // 16 + pos // 16).astype(np.int64)] = loc
        idx16 = np.tile(idx16, (8, 1))

        baryt = np.zeros((128, btot, 3), np.float32)
        baryt[g % 128, g // 128] = bary_slot[so]

        in_maps.append({
            "ftab": ftab,
            "idx16": idx16,
            "baryt": baryt.reshape(128, btot * 3),
        })
        unperm.append((so, g))

    key = caps
    if key not in _compiled_cache:
        _compiled_cache[key] = _build_program(caps)
    nc, btot_chk = _compiled_cache[key]
    assert btot_chk == btot

    from concourse.bass_utils import run_bass_kernel_spmd
    results = run_bass_kernel_spmd(nc, in_maps, list(range(NCORES))).results

    tex = np.empty((R, K, D), np.float32)
    pts = np.empty((R, K, 3), np.float32)
    nrm = np.empty((R, K, 3), np.float32)
    for c in range(NCORES):
        so, g = unperm[c]
        pr, bl = g % 128, g // 128
        texd = results[c]["texo"].reshape(128, btot, D)
        ptsd = results[c]["ptso"].reshape(128, btot, 3)
        nrmd = results[c]["nrmo"].reshape(128, btot, 3)
        texc = np.empty((SLOTS, D), np.float32)
        ptsc = np.empty((SLOTS, 3), np.float32)
        nrmc = np.empty((SLOTS, 3), np.float32)
        texc[so] = texd[pr, bl]
        ptsc[so] = ptsd[pr, bl]
        nrmc[so] = nrmd[pr, bl]
        sl = slice(c * RPC, (c + 1) * RPC)
        tex[sl] = texc.reshape(RPC, K, D)
        pts[sl] = ptsc.reshape(RPC, K, 3)
        nrm[sl] = nrmc.reshape(RPC, K, 3)

    depth = depth_s[pix].astype(np.float32)          # [R, K]
    return tex, pts, depth, nrm
